# revision 43
# baseline (speedup 1.0000x reference)
"""Trainium2 Bass kernel for AdditiveAttention (nn_AdditiveAttention_44564580663638).

Data-parallel over batch: B=8 -> one batch element per NeuronCore (8 cores),
no collectives. The [B,Q,K,H] tanh intermediate never touches HBM.

Per-core pipeline (Q=256, K=1024, D=256, H=128, DV=128):
  1. TensorE: project  A^T = W_q @ q^T  [H,Q],  B^T = W_k @ k^T  [H,K]
  2. VectorE: pre-add S[:,q,:] = B^T + A^T[:,q] (tensor_scalar per-partition
     add, bf16) for a group of G<=16 queries.
  3. ScalarE (the bottleneck, ~225us): one big tanh per group, FD = G*1024,
     amortizing the per-instruction overhead. Graded group sizes: tiny groups
     at kernel start (fast pipeline fill) and end (short epilogue lag).
  4. TensorE: score rows land in their PSUM partition via sliding-window
     one-hot weights: wvstrip[:, 32-r : 64-r] = w_v (x) e_r^T over a 32-col
     group, matmul with tile_position=(0, 32*(q//32)); 128 accumulating
     matmuls build the [128q, 1024k] score block in PSUM. The -1e6 softmax
     mask folds in as one extra K=1 matmul (ones (x) mask_row).
  5. ScalarE: E = exp(scores) straight out of PSUM, with Z = sum_k E from the
     activation's accum_out (free). No rowmax needed: |scores| <= sum|w_v|.
  6. TensorE: transpose raw E -> E^T chunks, accumulate out_raw = E^T.T @ V;
     the 1/Z softmax normalization is deferred to the final PSUM->SBUF copy
     (per-partition tensor_scalar_mul). Each block's epilogue is emitted
     behind the next block's first group so its DVE burst never stalls the
     pre-adds.
"""

import os
import sys

for _p in ("/opt/trn_rl_repo", "/root/.axon_site/_ro/trn_rl_repo"):
    if os.path.isdir(_p) and _p not in sys.path:
        sys.path.insert(0, _p)

import numpy as np
import ml_dtypes

import concourse.bass as bass
import concourse.bacc as bacc
import concourse.tile as tile
import concourse.mybir as mybir
from concourse.bass_utils import run_bass_kernel_spmd
from concourse.masks import make_identity

B, Q, K, DQ, DK, H, DV = 8, 256, 1024, 256, 256, 128, 128
P = 128
QB = Q // P      # query blocks of 128
KC = K // P      # key chunks of 128
DC = DQ // P     # contraction chunks for the projections
KH = K // 512    # psum half-banks per score row
FP32 = mybir.dt.float32
BF16 = mybir.dt.bfloat16
BF16_NP = ml_dtypes.bfloat16
NEG = -1e6

_NC = None
LAST_RESULT = None


def _build():
    nc = bacc.Bacc("TRN2", target_bir_lowering=False, debug=False)
    ACT = mybir.ActivationFunctionType

    qT_ext = nc.declare_dram_parameter("qT", [P, DC, Q], BF16, isOutput=False)
    kT_ext = nc.declare_dram_parameter("kT", [P, DC, K], BF16, isOutput=False)
    v_ext = nc.declare_dram_parameter("v", [P, KC, DV], BF16, isOutput=False)
    wqT_ext = nc.declare_dram_parameter("wqT", [P, DC, H], BF16, isOutput=False)
    wkT_ext = nc.declare_dram_parameter("wkT", [P, DC, H], BF16, isOutput=False)
    wv_ext = nc.declare_dram_parameter("wv", [H, 1], BF16, isOutput=False)
    mrow_ext = nc.declare_dram_parameter("mrow", [1, K], BF16, isOutput=False)
    out_ext = nc.declare_dram_parameter("out", [Q, DV], FP32, isOutput=True)

    with tile.TileContext(nc) as tc:
        with (
            tc.tile_pool(name="const", bufs=1) as constp,
            tc.tile_pool(name="sb", bufs=2) as sbp,
            tc.tile_pool(name="feat", bufs=3) as featp,
            tc.tile_pool(name="ps", bufs=2, space="PSUM") as psp,
            tc.tile_pool(name="ps2", bufs=2, space="PSUM") as psp2,
        ):
            # critical-path inputs first: kT/wk (for B^T), qT/wq (for A^T)
            kT_sb = constp.tile([P, DC, K], BF16)
            nc.sync.dma_start(kT_sb[:], kT_ext[:, :, :])
            wk_sb = constp.tile([P, DC, H], BF16)
            nc.sync.dma_start(wk_sb[:], wkT_ext[:, :, :])
            qT_sb = constp.tile([P, DC, Q], BF16)
            nc.sync.dma_start(qT_sb[:], qT_ext[:, :, :])
            wq_sb = constp.tile([P, DC, H], BF16)
            nc.sync.dma_start(wq_sb[:], wqT_ext[:, :, :])

            # projections (bf16 results: feed the DVE 4x-mode pre-adds)
            at_sb = constp.tile([H, Q], FP32)
            at_ps = psp.tile([H, Q], FP32, tag="sc")
            for c in range(DC):
                nc.tensor.matmul(
                    at_ps[:], wq_sb[:, c, :], qT_sb[:, c, :],
                    start=(c == 0), stop=(c == DC - 1),
                )
            nc.vector.tensor_copy(at_sb[:], at_ps[:])

            bt_sb = constp.tile([H, K], BF16)
            for kh in range(KH):
                bt_ps = psp.tile([H, 512], FP32, tag="sc")
                for c in range(DC):
                    nc.tensor.matmul(
                        bt_ps[:], wk_sb[:, c, :], kT_sb[:, c, kh * 512 : (kh + 1) * 512],
                        start=(c == 0), stop=(c == DC - 1),
                    )
                nc.vector.tensor_copy(bt_sb[:, kh * 512 : (kh + 1) * 512], bt_ps[:])

            # non-critical inputs after the projection chain
            mrow_sb = constp.tile([1, K], BF16)
            nc.sync.dma_start(mrow_sb[:], mrow_ext[:, :])
            v_sb = constp.tile([P, KC, DV], BF16)
            nc.sync.dma_start(v_sb[:], v_ext[:, :, :])
            ones_sb = constp.tile([1, P], BF16)
            nc.vector.memset(ones_sb[:], 1.0)
            ident = constp.tile([P, P], BF16)
            make_identity(nc, ident[:])
            # sliding-window one-hot w_v for 32-column weight loads:
            # wvstrip[:, 32:33] = w_v, zeros elsewhere; then
            # wvstrip[:, 32-r : 64-r] is w_v (x) e_r^T over a 32-col group.
            wv_sb = constp.tile([H, 1], BF16)
            nc.sync.dma_start(wv_sb[:], wv_ext[:, :])
            wvstrip = constp.tile([H, 65], BF16)
            nc.vector.memset(wvstrip[:], 0.0)
            nc.vector.tensor_copy(wvstrip[:, 32:33], wv_sb[:, :])

            pt_sb = constp.tile([P, KC, Q], BF16)  # P^T chunks [k, (kc, q)]

            # group sizes per 128-query block: small first groups (fast
            # pipeline fill at kernel start), small last groups (the final
            # score-matmul chain lags less behind the last tanh), large
            # interior groups (amortize ScalarE per-instruction overhead).
            GMAX = 16
            ramp_up = [1, 1, 2, 4, 8]
            ramp_dn = [8, 4, 2, 1, 1]

            def emit_epilogue(qb, sc_ps):
                # softmax (no rowmax needed: |scores| <= sum|w_v| ~ 11);
                # Z comes free via the activation's accumulator.
                e_sb = sbp.tile([P, KH, 512], BF16, tag="e")
                z_sb = sbp.tile([P, 1], FP32, tag="z")
                nc.scalar.activation(e_sb[:], sc_ps[:], ACT.Exp, accum_out=z_sb[:])
                r_sb = sbp.tile([P, 1], FP32, tag="r")
                nc.vector.reciprocal(r_sb[:], z_sb[:])
                # transpose raw E -> E^T chunks and accumulate the
                # unnormalized output; 1/Z is applied per-row at the final
                # PSUM->SBUF copy (deferred softmax normalization).
                o_ps = psp2.tile([P, DV], FP32, tag="o_ps")
                for kc in range(KC):
                    tp_ps = psp2.tile([P, P], BF16, tag="tp")
                    nc.tensor.transpose(
                        tp_ps[:],
                        e_sb[:, kc // 4, (kc % 4) * P : (kc % 4 + 1) * P],
                        ident[:],
                    )
                    nc.vector.tensor_copy(pt_sb[:, kc, qb * P : (qb + 1) * P], tp_ps[:])
                    nc.tensor.matmul(
                        o_ps[:], pt_sb[:, kc, qb * P : (qb + 1) * P], v_sb[:, kc, :],
                        start=(kc == 0), stop=(kc == KC - 1),
                    )
                o_sb = sbp.tile([P, DV], FP32, tag="o")
                nc.vector.tensor_scalar_mul(o_sb[:], o_ps[:], r_sb[:])
                nc.sync.dma_start(out_ext[qb * P : (qb + 1) * P, :], o_sb[:])

            interior = (P - sum(ramp_up)) // GMAX
            pending = None
            for qb in range(QB):
                sizes = (
                    ramp_up + [GMAX] * interior
                    if qb == 0
                    else [GMAX] * interior + ramp_dn
                )
                assert sum(sizes) == P
                sc_ps = psp.tile([P, KH, 512], FP32, tag="sc")
                q0 = 0
                for gi, gsz in enumerate(sizes):
                    # VectorE: pre-add B^T + A^T[:,q] per query (bf16 mode),
                    # then ScalarE tanh in place over the whole group.
                    s_t = featp.tile([H, GMAX, K], BF16, tag="s", bufs=3)
                    for j in range(gsz):
                        qg = qb * P + q0 + j
                        nc.vector.tensor_scalar_add(
                            s_t[:, j, :], bt_sb[:], at_sb[:, qg : qg + 1]
                        )
                    ft = featp.tile([H, GMAX, K], BF16, tag="ft", bufs=2)
                    nc.scalar.activation(
                        ft[:, 0:gsz, :], s_t[:, 0:gsz, :], ACT.Tanh
                    )
                    # TensorE: accumulate score rows into PSUM partitions.
                    # 32-col weight loads into the query's col-group: 4x
                    # cheaper LDWEIGHTS than a full 128-col load.
                    for j in range(gsz):
                        qi = q0 + j
                        cg, r = qi // 32, qi % 32
                        for kh in range(KH):
                            nc.tensor.matmul(
                                sc_ps[cg * 32 : (cg + 1) * 32, kh, :],
                                wvstrip[:, 32 - r : 64 - r],
                                ft[:, j, kh * 512 : (kh + 1) * 512],
                                start=(r == 0), stop=(qi == P - 1),
                                skip_group_check=True,
                                tile_position=(0, cg * 32),
                            )
                    if q0 <= 96 < q0 + gsz:
                        # additive -1e6 mask on every row: ones^T (x) mask_row.
                        # Must come after every 32-row col-group region has
                        # been started (qi=96 starts the last one).
                        for kh in range(KH):
                            nc.tensor.matmul(
                                sc_ps[:, kh, :], ones_sb[:],
                                mrow_sb[:, kh * 512 : (kh + 1) * 512],
                                start=False, stop=False, skip_group_check=True,
                            )
                    q0 += gsz
                    # previous block's epilogue rides behind this block's
                    # first group so its DVE burst doesn't stall the pre-adds
                    if gi == 0 and pending is not None:
                        emit_epilogue(*pending)
                        pending = None
                pending = (qb, sc_ps)
            emit_epilogue(*pending)

    nc.compile()
    return nc


def _get_nc():
    global _NC
    if _NC is None:
        _NC = _build()
    return _NC


def kernel(queries, keys, values, valid_lens, W_q, W_k, w_v):
    global LAST_RESULT
    queries = np.asarray(queries, dtype=np.float32)
    keys = np.asarray(keys, dtype=np.float32)
    values = np.asarray(values, dtype=np.float32)
    valid_lens = np.asarray(valid_lens, dtype=np.int32)
    W_q = np.asarray(W_q, dtype=np.float32)
    W_k = np.asarray(W_k, dtype=np.float32)
    w_v = np.asarray(w_v, dtype=np.float32)

    def pack(mat):
        # [C*P, F] -> [P, C, F]: partition-major so each SBUF partition's
        # data is one contiguous DRAM run (fast, few DMA descriptors)
        cp, f = mat.shape
        c = cp // P
        return np.ascontiguousarray(
            mat.reshape(c, P, f).transpose(1, 0, 2)
        ).astype(BF16_NP)

    wqT = pack(W_q.T)                                   # [P, DC, H]
    wkT = pack(W_k.T)                                   # [P, DC, H]
    wvc = np.ascontiguousarray(w_v[:, None]).astype(BF16_NP)  # [H, 1]
    ar = np.arange(K)

    in_maps = []
    for b in range(B):
        mrow = np.where(ar < int(valid_lens[b]), 0.0, NEG).astype(np.float32)
        in_maps.append({
            "qT": pack(queries[b].T),
            "kT": pack(keys[b].T),
            "v": pack(values[b]),
            "wqT": wqT,
            "wkT": wkT,
            "wv": wvc,
            "mrow": mrow[None, :].astype(BF16_NP),
            "out": np.zeros((Q, DV), dtype=np.float32),
        })

    nc = _get_nc()
    trace = bool(int(os.environ.get("KERNEL_TRACE", "0")))
    res = run_bass_kernel_spmd(nc, in_maps, core_ids=list(range(B)), trace=trace)
    LAST_RESULT = res
    out = np.stack([np.asarray(res.results[i]["out"], dtype=np.float32) for i in range(B)])
    return out


# revision 44
# speedup vs baseline: 1.0041x; 1.0041x over previous
"""Trainium2 Bass kernel for AdditiveAttention (nn_AdditiveAttention_44564580663638).

Data-parallel over batch: B=8 -> one batch element per NeuronCore (8 cores),
no collectives. The [B,Q,K,H] tanh intermediate never touches HBM.

Per-core pipeline (Q=256, K=1024, D=256, H=128, DV=128):
  1. TensorE: project  A^T = W_q @ q^T  [H,Q],  B^T = W_k @ k^T  [H,K]
  2. VectorE: pre-add S[:,q,:] = B^T + A^T[:,q] (tensor_scalar per-partition
     add, bf16) for a group of G<=16 queries.
  3. ScalarE (the bottleneck, ~225us): one big tanh per group, FD = G*1024,
     amortizing the per-instruction overhead. Graded group sizes: tiny groups
     at kernel start (fast pipeline fill) and end (short epilogue lag).
  4. TensorE: score rows land in their PSUM partition via sliding-window
     one-hot weights: wvstrip[:, 32-r : 64-r] = w_v (x) e_r^T over a 32-col
     group, matmul with tile_position=(0, 32*(q//32)); 128 accumulating
     matmuls build the [128q, 1024k] score block in PSUM. The -1e6 softmax
     mask folds in as one extra K=1 matmul (ones (x) mask_row).
  5. ScalarE: E = exp(scores) straight out of PSUM, with Z = sum_k E from the
     activation's accum_out (free). No rowmax needed: |scores| <= sum|w_v|.
  6. TensorE: transpose raw E -> E^T chunks, accumulate out_raw = E^T.T @ V;
     the 1/Z softmax normalization is deferred to the final PSUM->SBUF copy
     (per-partition tensor_scalar_mul). Each block's epilogue is emitted
     behind the next block's first group so its DVE burst never stalls the
     pre-adds.
"""

import os
import sys

for _p in ("/opt/trn_rl_repo", "/root/.axon_site/_ro/trn_rl_repo"):
    if os.path.isdir(_p) and _p not in sys.path:
        sys.path.insert(0, _p)

import numpy as np
import ml_dtypes

import concourse.bass as bass
import concourse.bacc as bacc
import concourse.tile as tile
import concourse.mybir as mybir
from concourse.bass_utils import run_bass_kernel_spmd
from concourse.masks import make_identity

B, Q, K, DQ, DK, H, DV = 8, 256, 1024, 256, 256, 128, 128
P = 128
QB = Q // P      # query blocks of 128
KC = K // P      # key chunks of 128
DC = DQ // P     # contraction chunks for the projections
KH = K // 512    # psum half-banks per score row
FP32 = mybir.dt.float32
BF16 = mybir.dt.bfloat16
BF16_NP = ml_dtypes.bfloat16
NEG = -1e6

_NC = None
LAST_RESULT = None


def _build():
    nc = bacc.Bacc("TRN2", target_bir_lowering=False, debug=False)
    ACT = mybir.ActivationFunctionType

    qT_ext = nc.declare_dram_parameter("qT", [P, DC, Q], BF16, isOutput=False)
    kT_ext = nc.declare_dram_parameter("kT", [P, DC, K], BF16, isOutput=False)
    v_ext = nc.declare_dram_parameter("v", [P, KC, DV], BF16, isOutput=False)
    wqT_ext = nc.declare_dram_parameter("wqT", [P, DC, H], BF16, isOutput=False)
    wkT_ext = nc.declare_dram_parameter("wkT", [P, DC, H], BF16, isOutput=False)
    wv_ext = nc.declare_dram_parameter("wv", [H, 1], BF16, isOutput=False)
    mrow_ext = nc.declare_dram_parameter("mrow", [1, K], BF16, isOutput=False)
    out_ext = nc.declare_dram_parameter("out", [Q, DV], FP32, isOutput=True)

    with tile.TileContext(nc) as tc:
        with (
            tc.tile_pool(name="const", bufs=1) as constp,
            tc.tile_pool(name="sb", bufs=2) as sbp,
            tc.tile_pool(name="feat", bufs=3) as featp,
            tc.tile_pool(name="ps", bufs=2, space="PSUM") as psp,
            tc.tile_pool(name="ps2", bufs=2, space="PSUM") as psp2,
        ):
            # critical-path inputs first: kT/wk (for B^T), qT/wq (for A^T)
            kT_sb = constp.tile([P, DC, K], BF16)
            nc.sync.dma_start(kT_sb[:], kT_ext[:, :, :])
            wk_sb = constp.tile([P, DC, H], BF16)
            nc.sync.dma_start(wk_sb[:], wkT_ext[:, :, :])
            qT_sb = constp.tile([P, DC, Q], BF16)
            nc.sync.dma_start(qT_sb[:], qT_ext[:, :, :])
            wq_sb = constp.tile([P, DC, H], BF16)
            nc.sync.dma_start(wq_sb[:], wqT_ext[:, :, :])

            # projections (bf16 results: feed the DVE 4x-mode pre-adds)
            at_sb = constp.tile([H, Q], FP32)
            at_ps = psp.tile([H, Q], FP32, tag="sc")
            for c in range(DC):
                nc.tensor.matmul(
                    at_ps[:], wq_sb[:, c, :], qT_sb[:, c, :],
                    start=(c == 0), stop=(c == DC - 1),
                )
            nc.vector.tensor_copy(at_sb[:], at_ps[:])

            bt_sb = constp.tile([H, K], BF16)
            for kh in range(KH):
                bt_ps = psp.tile([H, 512], FP32, tag="sc")
                for c in range(DC):
                    nc.tensor.matmul(
                        bt_ps[:], wk_sb[:, c, :], kT_sb[:, c, kh * 512 : (kh + 1) * 512],
                        start=(c == 0), stop=(c == DC - 1),
                    )
                nc.vector.tensor_copy(bt_sb[:, kh * 512 : (kh + 1) * 512], bt_ps[:])

            # non-critical inputs after the projection chain
            mrow_sb = constp.tile([1, K], BF16)
            nc.sync.dma_start(mrow_sb[:], mrow_ext[:, :])
            v_sb = constp.tile([P, KC, DV], BF16)
            nc.sync.dma_start(v_sb[:], v_ext[:, :, :])
            ones_sb = constp.tile([1, P], BF16)
            nc.vector.memset(ones_sb[:], 1.0)
            ident = constp.tile([P, P], BF16)
            make_identity(nc, ident[:])
            # sliding-window one-hot w_v for 32-column weight loads:
            # wvstrip[:, 32:33] = w_v, zeros elsewhere; then
            # wvstrip[:, 32-r : 64-r] is w_v (x) e_r^T over a 32-col group.
            wv_sb = constp.tile([H, 1], BF16)
            nc.sync.dma_start(wv_sb[:], wv_ext[:, :])
            wvstrip = constp.tile([H, 65], BF16)
            nc.vector.memset(wvstrip[:], 0.0)
            nc.vector.tensor_copy(wvstrip[:, 32:33], wv_sb[:, :])

            pt_sb = constp.tile([P, KC, Q], BF16)  # P^T chunks [k, (kc, q)]

            # group sizes per 128-query block: small first groups (fast
            # pipeline fill at kernel start), small last groups (the final
            # score-matmul chain lags less behind the last tanh), large
            # interior groups (amortize ScalarE per-instruction overhead).
            GMAX = 16
            ramp_up = [1, 1, 2, 4, 8]
            ramp_dn = [8, 4, 2, 1, 1]

            def emit_epilogue(qb, sc_ps):
                # softmax (no rowmax needed: |scores| <= sum|w_v| ~ 11);
                # Z comes free via the activation's accumulator.
                e_sb = sbp.tile([P, KH, 512], BF16, tag="e")
                z_sb = sbp.tile([P, 1], FP32, tag="z")
                nc.scalar.activation(e_sb[:], sc_ps[:], ACT.Exp, accum_out=z_sb[:])
                r_sb = sbp.tile([P, 1], FP32, tag="r")
                nc.vector.reciprocal(r_sb[:], z_sb[:])
                # transpose raw E -> E^T chunks and accumulate the
                # unnormalized output; 1/Z is applied per-row at the final
                # PSUM->SBUF copy (deferred softmax normalization).
                o_ps = psp2.tile([P, DV], FP32, tag="o_ps")
                for kc in range(KC):
                    tp_ps = psp2.tile([P, P], BF16, tag="tp")
                    nc.tensor.transpose(
                        tp_ps[:],
                        e_sb[:, kc // 4, (kc % 4) * P : (kc % 4 + 1) * P],
                        ident[:],
                    )
                    nc.vector.tensor_copy(pt_sb[:, kc, qb * P : (qb + 1) * P], tp_ps[:])
                    nc.tensor.matmul(
                        o_ps[:], pt_sb[:, kc, qb * P : (qb + 1) * P], v_sb[:, kc, :],
                        start=(kc == 0), stop=(kc == KC - 1),
                    )
                o_sb = sbp.tile([P, DV], FP32, tag="o")
                nc.vector.tensor_scalar_mul(o_sb[:], o_ps[:], r_sb[:])
                nc.sync.dma_start(out_ext[qb * P : (qb + 1) * P, :], o_sb[:])

            interior = (P - sum(ramp_up)) // GMAX
            pending = None
            for qb in range(QB):
                sizes = (
                    ramp_up + [GMAX] * interior
                    if qb == 0
                    else [GMAX] * interior + ramp_dn
                )
                assert sum(sizes) == P
                sc_ps = psp.tile([P, KH, 512], FP32, tag="sc")
                q0 = 0
                for gi, gsz in enumerate(sizes):
                    # VectorE: pre-add B^T + A^T[:,q] per query (bf16 mode),
                    # then ScalarE tanh in place over the whole group.
                    s_t = featp.tile([H, GMAX, K], BF16, tag="s", bufs=2)
                    for j in range(gsz):
                        qg = qb * P + q0 + j
                        nc.vector.tensor_scalar_add(
                            s_t[:, j, :], bt_sb[:], at_sb[:, qg : qg + 1]
                        )
                    ft = featp.tile([H, GMAX, K], BF16, tag="ft", bufs=3)
                    nc.scalar.activation(
                        ft[:, 0:gsz, :], s_t[:, 0:gsz, :], ACT.Tanh
                    )
                    # TensorE: accumulate score rows into PSUM partitions.
                    # 32-col weight loads into the query's col-group: 4x
                    # cheaper LDWEIGHTS than a full 128-col load.
                    for j in range(gsz):
                        qi = q0 + j
                        cg, r = qi // 32, qi % 32
                        for kh in range(KH):
                            nc.tensor.matmul(
                                sc_ps[cg * 32 : (cg + 1) * 32, kh, :],
                                wvstrip[:, 32 - r : 64 - r],
                                ft[:, j, kh * 512 : (kh + 1) * 512],
                                start=(r == 0), stop=(qi == P - 1),
                                skip_group_check=True,
                                tile_position=(0, cg * 32),
                            )
                    if q0 <= 96 < q0 + gsz:
                        # additive -1e6 mask on every row: ones^T (x) mask_row.
                        # Must come after every 32-row col-group region has
                        # been started (qi=96 starts the last one).
                        for kh in range(KH):
                            nc.tensor.matmul(
                                sc_ps[:, kh, :], ones_sb[:],
                                mrow_sb[:, kh * 512 : (kh + 1) * 512],
                                start=False, stop=False, skip_group_check=True,
                            )
                    q0 += gsz
                    # previous block's epilogue rides behind this block's
                    # first group so its DVE burst doesn't stall the pre-adds
                    if gi == 0 and pending is not None:
                        emit_epilogue(*pending)
                        pending = None
                pending = (qb, sc_ps)
            emit_epilogue(*pending)

    nc.compile()
    return nc


def _get_nc():
    global _NC
    if _NC is None:
        _NC = _build()
    return _NC


def kernel(queries, keys, values, valid_lens, W_q, W_k, w_v):
    global LAST_RESULT
    queries = np.asarray(queries, dtype=np.float32)
    keys = np.asarray(keys, dtype=np.float32)
    values = np.asarray(values, dtype=np.float32)
    valid_lens = np.asarray(valid_lens, dtype=np.int32)
    W_q = np.asarray(W_q, dtype=np.float32)
    W_k = np.asarray(W_k, dtype=np.float32)
    w_v = np.asarray(w_v, dtype=np.float32)

    def pack(mat):
        # [C*P, F] -> [P, C, F]: partition-major so each SBUF partition's
        # data is one contiguous DRAM run (fast, few DMA descriptors)
        cp, f = mat.shape
        c = cp // P
        return np.ascontiguousarray(
            mat.reshape(c, P, f).transpose(1, 0, 2)
        ).astype(BF16_NP)

    wqT = pack(W_q.T)                                   # [P, DC, H]
    wkT = pack(W_k.T)                                   # [P, DC, H]
    wvc = np.ascontiguousarray(w_v[:, None]).astype(BF16_NP)  # [H, 1]
    ar = np.arange(K)

    in_maps = []
    for b in range(B):
        mrow = np.where(ar < int(valid_lens[b]), 0.0, NEG).astype(np.float32)
        in_maps.append({
            "qT": pack(queries[b].T),
            "kT": pack(keys[b].T),
            "v": pack(values[b]),
            "wqT": wqT,
            "wkT": wkT,
            "wv": wvc,
            "mrow": mrow[None, :].astype(BF16_NP),
            "out": np.zeros((Q, DV), dtype=np.float32),
        })

    nc = _get_nc()
    trace = bool(int(os.environ.get("KERNEL_TRACE", "0")))
    res = run_bass_kernel_spmd(nc, in_maps, core_ids=list(range(B)), trace=trace)
    LAST_RESULT = res
    out = np.stack([np.asarray(res.results[i]["out"], dtype=np.float32) for i in range(B)])
    return out
